# revision 27
# baseline (speedup 1.0000x reference)
"""TRN2 Bass kernel for CompressedLinearLayer: out = x @ (A @ B.T).T + bias.

Computed low-rank: t = x @ B  (rank 512), out = t @ A.T + bias.
Sharding: data-parallel over the 8192 rows of x (1024 rows per core);
B, A.T, bias replicated. No collectives.

Device layouts (per core), bf16 on the wire for matmul inputs:
  xT   [4096, 1024] bf16  x rows shard, transposed+converted on host
  b    [4096, 512]  bf16  B
  at   [512, 4096]  bf16  A.T
  bias [4096]       f32
  out  [1024, 4096] bf16  natural orientation (host upconverts to f32)

Schedule (per core), rows in 2 blocks of 512:
  PE order: warmup -> s1(b0) g0..g7 -> s1(b1) g0..g3 -> s2(b0) dch0 units
            -> s1(b1) g4..g7 -> s2(b0) dch1 units -> s2(b1) all units.
  DMA wire order on the sync ring: x(b0) g0..g7, x(b1) g0..g3,
  A.T half0, x(b1) g4..g7, A.T half1.  B co-streams on the vector ring
  during phase A so each stage-1 group's B and x chunks land together.
  Output stores alternate between the scalar and gpsimd rings.
  Stage-2 units do the last k-chunk dc-major with immediate per-psum
  bias-add evacuation so PSUM bank turnaround hides behind the next
  unit's matmuls.  Accumulation is fp32 in PSUM; bias is added during
  PSUM evacuation on the vector engine; output stored as bf16.
"""
import numpy as np
import ml_dtypes

import concourse.bacc as bacc
import concourse.mybir as mybir
import concourse.tile as tile
from concourse.bass_utils import run_bass_kernel_spmd

N_CORES = 8
BATCH, SEQ = 4, 2048
D_IN, D_OUT, RANK = 4096, 4096, 512
ROWS_TOTAL = BATCH * SEQ           # 8192
ROWS = ROWS_TOTAL // N_CORES       # 1024 rows per core

F32 = mybir.dt.float32
BF16 = mybir.dt.bfloat16

KC = D_IN // 128     # 32 contraction chunks, stage 1
KSUB = 4             # k-chunks packed per DMA (0.5MB bf16 transfers)
KB = KC // KSUB      # 8 packed k-groups per block
RC = RANK // 128     # 4 rank chunks
NBLK = 2             # row blocks per core
BROWS = ROWS // NBLK # 512 rows per block
MB2 = BROWS // 128   # 4 row chunks of 128 per block (stage-2 out partitions)
N_WARM = 14          # PE warmup matmuls (ramp pstate during first DMA wait)

_compiled = {}


def _build():
    nc = bacc.Bacc("TRN2", target_bir_lowering=False, debug=False)

    xT_d = nc.declare_dram_parameter("xT", [D_IN, ROWS], BF16, isOutput=False)
    b_d = nc.declare_dram_parameter("b", [D_IN, RANK], BF16, isOutput=False)
    at_d = nc.declare_dram_parameter("at", [RANK, D_OUT], BF16, isOutput=False)
    bias_d = nc.declare_dram_parameter("bias", [D_OUT], F32, isOutput=False)
    out_d = nc.declare_dram_parameter("out", [ROWS, D_OUT], BF16, isOutput=True)

    with tile.TileContext(nc) as tc:
        with (
            tc.tile_pool(name="wb", bufs=1) as wb,
            tc.tile_pool(name="xp", bufs=8) as xp,
            tc.tile_pool(name="tt", bufs=1) as ttp,
            tc.tile_pool(name="op", bufs=6) as op,
            tc.tile_pool(name="ps1", bufs=4, space="PSUM") as ps1p,
            tc.tile_pool(name="ps2", bufs=4, space="PSUM") as ps2p,
        ):
            bias_bc = wb.tile([128, D_OUT], F32, tag="bias_bc")

            # B resident: 8 tiles [128, 4, 512] bf16 = 0.5MB each
            b_sb = [
                wb.tile([128, KSUB, RANK], BF16, tag=f"b{g}", name=f"b{g}")
                for g in range(KB)
            ]
            # A.T resident: 4 tiles [128, 4096] bf16 (1MB each)
            at_sb = [
                wb.tile([128, D_OUT], BF16, tag=f"at{r}", name=f"at{r}")
                for r in range(RC)
            ]
            # tT per block: 4 tiles [128, 512] bf16 each
            tT = [
                [
                    ttp.tile([128, BROWS], BF16, tag=f"tT{b}_{r}", name=f"tT{b}_{r}")
                    for r in range(RC)
                ]
                for b in range(NBLK)
            ]
            # warmup scratch
            wstat = wb.tile([128, 128], BF16, tag="wstat")
            wmov = wb.tile([128, 512], BF16, tag="wmov")
            wps_ref = []

            def warmup():
                nc.vector.memset(wstat[:], 0.0)
                nc.vector.memset(wmov[:], 0.0)
                wps = ps2p.tile([128, 512], F32, tag="ps2", name="warm_ps")
                wps_ref.append(wps)
                for i in range(N_WARM):
                    nc.tensor.matmul(
                        wps[:], wstat[:], wmov[:], start=True, stop=True
                    )

            def filler(n):
                # keep the PE busy (and its pstate up) across a DMA-arrival
                # jitter window; results are never read
                wps = wps_ref[0]
                for i in range(n):
                    nc.tensor.matmul(
                        wps[:], wstat[:], wmov[:], start=True, stop=True
                    )

            def load_bias():
                nc.sync.dma_start(bias_bc[0:1, :], bias_d[None, :])
                nc.gpsimd.partition_broadcast(bias_bc[:], bias_bc[0:1, :])

            def stage1_group(b, g, psum1):
                # DMAs: x chunk on sync ring; B chunk (block 0 only) on
                # vector ring so both streams co-arrive in phase A.
                # phase-A arrival granularity: quarter chunks while the PE
                # has no DMA lead yet, coarsening as the lead builds
                split = 2 if (b == 0 and g < 3) else 1
                xq = nc.sync
                xg = xp.tile([128, KSUB, BROWS], BF16, tag="xk", name=f"x{b}_{g}")
                for sp in range(split):
                    lo, hi = sp * KSUB // split, (sp + 1) * KSUB // split
                    if b == 0:
                        nc.scalar.dma_start(
                            b_sb[g][:, lo:hi, :],
                            b_d[(g * KSUB + lo) * 128:(g * KSUB + hi) * 128, :]
                            .rearrange("(ks p) r -> p ks r", p=128),
                        )
                    xq.dma_start(
                        xg[:, lo:hi, :],
                        xT_d[
                            (g * KSUB + lo) * 128:(g * KSUB + hi) * 128,
                            b * BROWS:(b + 1) * BROWS,
                        ].rearrange("(ks p) m -> p ks m", p=128),
                    )
                last = g == KB - 1
                if not last:
                    for ks in range(KSUB):
                        k = g * KSUB + ks
                        for mc in range(RC):
                            nc.tensor.matmul(
                                psum1[mc][:],
                                b_sb[g][:, ks, mc * 128:(mc + 1) * 128],
                                xg[:, ks, :],
                                start=(k == 0),
                                stop=False,
                            )
                else:
                    # invert loops so each psum finishes (and can evacuate to
                    # tT on the DVE) while the PE continues with the next mc
                    for mc in range(RC):
                        for ks in range(KSUB):
                            nc.tensor.matmul(
                                psum1[mc][:],
                                b_sb[g][:, ks, mc * 128:(mc + 1) * 128],
                                xg[:, ks, :],
                                start=False,
                                stop=(ks == KSUB - 1),
                            )
                        nc.vector.tensor_copy(tT[b][mc][:], psum1[mc][:])

            def stage1_psum(b):
                return [
                    ps1p.tile([128, BROWS], F32, tag="ps1", name=f"ps1_{b}_{i}")
                    for i in range(RC)
                ]

            def load_at_half(h):
                # sync ring: wire-ordered behind the x chunks emitted so far
                for r in range(RC):
                    nc.sync.dma_start(
                        at_sb[r][:, h * 2048:(h + 1) * 2048],
                        at_d[r * 128:(r + 1) * 128, h * 2048:(h + 1) * 2048],
                    )

            _store_flip = [0]

            def _store(dram_ap, sbuf_ap, alternate=False):
                eng = nc.scalar
                if alternate:
                    # sync ring is idle by the time the final stores go out
                    eng = nc.scalar if _store_flip[0] == 0 else nc.sync
                    _store_flip[0] ^= 1
                eng.dma_start(dram_ap, sbuf_ap)

            def stage2_unit(b, rc2, dch, fine_stores=False):
                row0 = rc2 * 128
                psum2 = [
                    ps2p.tile(
                        [128, 512], F32, tag="ps2",
                        name=f"ps2_{b}_{rc2}_{dch}_{i}",
                    )
                    for i in range(4)
                ]
                # k = 0..RC-2: dc-inner
                for k in range(RC - 1):
                    for dc in range(4):
                        d0 = (dch * 4 + dc) * 512
                        nc.tensor.matmul(
                            psum2[dc][:],
                            tT[b][k][:, row0:row0 + 128],
                            at_sb[k][:, d0:d0 + 512],
                            start=(k == 0),
                            stop=False,
                        )
                # last k: dc-major with immediate evacuation so each psum
                # bank frees while the PE continues with the next dc
                ot = [
                    op.tile([128, 1024], BF16, tag="ot",
                            name=f"ot{b}_{rc2}_{dch}_{j}")
                    for j in range(2)
                ]
                for dc in range(4):
                    d0 = (dch * 4 + dc) * 512
                    nc.tensor.matmul(
                        psum2[dc][:],
                        tT[b][RC - 1][:, row0:row0 + 128],
                        at_sb[RC - 1][:, d0:d0 + 512],
                        start=False,
                        stop=True,
                    )
                    j, half = dc // 2, dc % 2
                    nc.vector.tensor_add(
                        ot[j][:, half * 512:(half + 1) * 512],
                        psum2[dc][:],
                        bias_bc[:, d0:d0 + 512],
                    )
                    if fine_stores:
                        _store(
                            out_d[
                                b * BROWS + row0:b * BROWS + row0 + 128,
                                d0:d0 + 512,
                            ],
                            ot[j][:, half * 512:(half + 1) * 512],
                            alternate=True,
                        )
                    elif half == 1:
                        _store(
                            out_d[
                                b * BROWS + row0:b * BROWS + row0 + 128,
                                dch * 2048 + j * 1024:dch * 2048 + (j + 1) * 1024,
                            ],
                            ot[j][:],
                        )

            warmup()

            # stage1 block 0 (B + x co-stream); small PE fillers bridge
            # wire jitter at the early chunk boundaries
            ps_a = stage1_psum(0)
            for g in range(KB):
                stage1_group(0, g, ps_a)
                if g < 4:
                    filler(2)
            load_bias()

            # stage1 block 1 first half (x(b1) g0..g3 ahead of A.T on the wire)
            ps_b = stage1_psum(1)
            for g in range(4):
                stage1_group(1, g, ps_b)

            # stage2 block 0, dch=0 units (A.T half0 lands behind x(b1) g0..g3)
            load_at_half(0)
            for rc2 in range(MB2):
                stage2_unit(0, rc2, 0)

            # stage1 block 1 second half
            for g in range(4, KB):
                stage1_group(1, g, ps_b)

            # stage2 block 0, dch=1 units
            load_at_half(1)
            for rc2 in range(MB2):
                stage2_unit(0, rc2, 1)

            # stage2 block 1
            for rc2 in range(MB2):
                for dch in range(2):
                    stage2_unit(
                        1, rc2, dch,
                        fine_stores=(rc2 == MB2 - 1 and dch == 1),
                    )

    nc.compile()
    return nc


def _get_nc():
    if "nc" not in _compiled:
        _compiled["nc"] = _build()
    return _compiled["nc"]


def run(inputs, trace=False, trace_kwargs=None):
    """Shard, execute on 8 cores, gather. Returns (output, BassKernelResults)."""
    x = np.asarray(inputs["x"], dtype=np.float32)
    A = np.asarray(inputs["A"], dtype=np.float32)
    B = np.asarray(inputs["B"], dtype=np.float32)
    bias = np.asarray(inputs["bias"], dtype=np.float32)

    x_flat = x.reshape(ROWS_TOTAL, D_IN)
    B_bf = B.astype(ml_dtypes.bfloat16)
    AT_bf = np.ascontiguousarray(A.T).astype(ml_dtypes.bfloat16)
    in_maps = []
    for i in range(N_CORES):
        xT_i = np.ascontiguousarray(x_flat[i * ROWS:(i + 1) * ROWS].T).astype(
            ml_dtypes.bfloat16
        )
        in_maps.append({"xT": xT_i, "b": B_bf, "at": AT_bf, "bias": bias})

    nc = _get_nc()
    kwargs = {}
    if trace:
        kwargs["trace"] = True
        kwargs["trace_kwargs"] = trace_kwargs or {}
    res = None
    for attempt in range(3):
        try:
            res = run_bass_kernel_spmd(
                nc, in_maps, core_ids=list(range(N_CORES)), **kwargs
            )
        except Exception:
            # transient device/runtime hiccup; retry
            if attempt == 2:
                raise
            continue
        out = np.concatenate(
            [
                res.results[i]["out"].astype(np.float32)
                for i in range(N_CORES)
            ],
            axis=0,
        )
        if np.isfinite(out).all():
            return out.reshape(BATCH, SEQ, D_OUT), res
    return out.reshape(BATCH, SEQ, D_OUT), res


def kernel(**inputs) -> np.ndarray:
    out, _ = run(inputs)
    return out


# revision 35
# speedup vs baseline: 1.0001x; 1.0001x over previous
"""TRN2 Bass kernel for CompressedLinearLayer: out = x @ (A @ B.T).T + bias.

Computed low-rank: t = x @ B  (rank 512), out = t @ A.T + bias.
Sharding: data-parallel over the 8192 rows of x (1024 rows per core);
B, A.T, bias replicated. No collectives.

Device layouts (per core), bf16 on the wire for matmul inputs:
  xT   [4096, 1024] bf16  x rows shard, transposed+converted on host
  b    [4096, 512]  bf16  B
  at   [512, 4096]  bf16  A.T
  bias [4096]       f32
  out  [1024, 4096] bf16  natural orientation (host upconverts to f32)

Schedule (per core), rows in 2 blocks of 512:
  PE order: warmup -> s1(b0) g0..g7 -> s1(b1) g0..g3 -> s2(b0) dch0 units
            -> s1(b1) g4..g7 -> s2(b0) dch1 units -> s2(b1) all units.
  DMA wire order on the sync ring: x(b0) g0..g7, x(b1) g0..g3,
  A.T half0, x(b1) g4..g7, A.T half1.  B co-streams on the vector ring
  during phase A so each stage-1 group's B and x chunks land together.
  Output stores alternate between the scalar and gpsimd rings.
  Stage-2 units do the last k-chunk dc-major with immediate per-psum
  bias-add evacuation so PSUM bank turnaround hides behind the next
  unit's matmuls.  Accumulation is fp32 in PSUM; bias is added during
  PSUM evacuation on the vector engine; output stored as bf16.
"""
import numpy as np
import ml_dtypes

import concourse.bacc as bacc
import concourse.mybir as mybir
import concourse.tile as tile
from concourse.bass_utils import run_bass_kernel_spmd

N_CORES = 8
BATCH, SEQ = 4, 2048
D_IN, D_OUT, RANK = 4096, 4096, 512
ROWS_TOTAL = BATCH * SEQ           # 8192
ROWS = ROWS_TOTAL // N_CORES       # 1024 rows per core

F32 = mybir.dt.float32
BF16 = mybir.dt.bfloat16

KC = D_IN // 128     # 32 contraction chunks, stage 1
KSUB = 4             # k-chunks packed per DMA (0.5MB bf16 transfers)
KB = KC // KSUB      # 8 packed k-groups per block
RC = RANK // 128     # 4 rank chunks
NBLK = 2             # row blocks per core
BROWS = ROWS // NBLK # 512 rows per block
MB2 = BROWS // 128   # 4 row chunks of 128 per block (stage-2 out partitions)
N_WARM = 14          # PE warmup matmuls (ramp pstate during first DMA wait)

_compiled = {}


def _build():
    nc = bacc.Bacc("TRN2", target_bir_lowering=False, debug=False)

    xT_d = nc.declare_dram_parameter("xT", [D_IN, ROWS], BF16, isOutput=False)
    b_d = nc.declare_dram_parameter("b", [D_IN, RANK], BF16, isOutput=False)
    at_d = nc.declare_dram_parameter("at", [RANK, D_OUT], BF16, isOutput=False)
    bias_d = nc.declare_dram_parameter("bias", [128, D_OUT], BF16, isOutput=False)
    out_d = nc.declare_dram_parameter("out", [ROWS, D_OUT], BF16, isOutput=True)

    with tile.TileContext(nc) as tc:
        with (
            tc.tile_pool(name="wb", bufs=1) as wb,
            tc.tile_pool(name="xp", bufs=8) as xp,
            tc.tile_pool(name="tt", bufs=1) as ttp,
            tc.tile_pool(name="op", bufs=6) as op,
            tc.tile_pool(name="ps1", bufs=4, space="PSUM") as ps1p,
            tc.tile_pool(name="ps2", bufs=4, space="PSUM") as ps2p,
        ):
            bias_bc = wb.tile([128, D_OUT], BF16, tag="bias_bc")

            # B resident: 8 tiles [128, 4, 512] bf16 = 0.5MB each
            b_sb = [
                wb.tile([128, KSUB, RANK], BF16, tag=f"b{g}", name=f"b{g}")
                for g in range(KB)
            ]
            # A.T resident: 4 tiles [128, 4096] bf16 (1MB each)
            at_sb = [
                wb.tile([128, D_OUT], BF16, tag=f"at{r}", name=f"at{r}")
                for r in range(RC)
            ]
            # tT per block: 4 tiles [128, 512] bf16 each
            tT = [
                [
                    ttp.tile([128, BROWS], BF16, tag=f"tT{b}_{r}", name=f"tT{b}_{r}")
                    for r in range(RC)
                ]
                for b in range(NBLK)
            ]
            # warmup scratch
            wstat = wb.tile([128, 128], BF16, tag="wstat")
            wmov = wb.tile([128, 512], BF16, tag="wmov")
            wps_ref = []

            def warmup():
                nc.vector.memset(wstat[:], 0.0)
                nc.vector.memset(wmov[:], 0.0)
                wps = ps2p.tile([128, 512], F32, tag="ps2", name="warm_ps")
                wps_ref.append(wps)
                for i in range(N_WARM):
                    nc.tensor.matmul(
                        wps[:], wstat[:], wmov[:], start=True, stop=True
                    )

            def filler(n):
                # keep the PE busy (and its pstate up) across a DMA-arrival
                # jitter window; results are never read
                wps = wps_ref[0]
                for i in range(n):
                    nc.tensor.matmul(
                        wps[:], wstat[:], wmov[:], start=True, stop=True
                    )

            def load_bias():
                # bias pre-replicated to 128 partitions on the host (bf16);
                # rides the scalar ring behind B, needed only from ~50us
                nc.scalar.dma_start(bias_bc[:], bias_d[:])

            def stage1_group(b, g, psum1):
                # DMAs: x chunk on sync ring; B chunk (block 0 only) on
                # vector ring so both streams co-arrive in phase A.
                # phase-A arrival granularity: quarter chunks while the PE
                # has no DMA lead yet, coarsening as the lead builds
                split = 2 if (b == 0 and g < 3) else 1
                xq = nc.sync
                xg = xp.tile([128, KSUB, BROWS], BF16, tag="xk", name=f"x{b}_{g}")
                for sp in range(split):
                    lo, hi = sp * KSUB // split, (sp + 1) * KSUB // split
                    if b == 0:
                        nc.scalar.dma_start(
                            b_sb[g][:, lo:hi, :],
                            b_d[(g * KSUB + lo) * 128:(g * KSUB + hi) * 128, :]
                            .rearrange("(ks p) r -> p ks r", p=128),
                        )
                    xq.dma_start(
                        xg[:, lo:hi, :],
                        xT_d[
                            (g * KSUB + lo) * 128:(g * KSUB + hi) * 128,
                            b * BROWS:(b + 1) * BROWS,
                        ].rearrange("(ks p) m -> p ks m", p=128),
                    )
                last = g == KB - 1
                if not last:
                    for ks in range(KSUB):
                        k = g * KSUB + ks
                        for mc in range(RC):
                            nc.tensor.matmul(
                                psum1[mc][:],
                                b_sb[g][:, ks, mc * 128:(mc + 1) * 128],
                                xg[:, ks, :],
                                start=(k == 0),
                                stop=False,
                            )
                else:
                    # invert loops so each psum finishes (and can evacuate to
                    # tT on the DVE) while the PE continues with the next mc
                    for mc in range(RC):
                        for ks in range(KSUB):
                            nc.tensor.matmul(
                                psum1[mc][:],
                                b_sb[g][:, ks, mc * 128:(mc + 1) * 128],
                                xg[:, ks, :],
                                start=False,
                                stop=(ks == KSUB - 1),
                            )
                        nc.vector.tensor_copy(tT[b][mc][:], psum1[mc][:])

            def stage1_psum(b):
                return [
                    ps1p.tile([128, BROWS], F32, tag="ps1", name=f"ps1_{b}_{i}")
                    for i in range(RC)
                ]

            def load_at_half(h):
                # sync ring: wire-ordered behind the x chunks emitted so far
                for r in range(RC):
                    nc.sync.dma_start(
                        at_sb[r][:, h * 2048:(h + 1) * 2048],
                        at_d[r * 128:(r + 1) * 128, h * 2048:(h + 1) * 2048],
                    )

            _store_flip = [0]

            def _store(dram_ap, sbuf_ap, alternate=False):
                eng = nc.scalar
                if alternate:
                    # sync ring is idle by the time the final stores go out
                    eng = nc.scalar if _store_flip[0] == 0 else nc.sync
                    _store_flip[0] ^= 1
                eng.dma_start(dram_ap, sbuf_ap)

            def stage2_unit(b, rc2, dch, dcs=(0, 1, 2, 3), fine_stores=False):
                row0 = rc2 * 128
                psum2 = {
                    dc: ps2p.tile(
                        [128, 512], F32, tag="ps2",
                        name=f"ps2_{b}_{rc2}_{dch}_{dc}",
                    )
                    for dc in dcs
                }
                # k = 0..RC-2: dc-inner
                for k in range(RC - 1):
                    for dc in dcs:
                        d0 = (dch * 4 + dc) * 512
                        nc.tensor.matmul(
                            psum2[dc][:],
                            tT[b][k][:, row0:row0 + 128],
                            at_sb[k][:, d0:d0 + 512],
                            start=(k == 0),
                            stop=False,
                        )
                # last k: dc-major with immediate evacuation so each psum
                # bank frees while the PE continues with the next dc
                ot = {
                    j: op.tile([128, 1024], BF16, tag="ot",
                               name=f"ot{b}_{rc2}_{dch}_{j}")
                    for j in set(dc // 2 for dc in dcs)
                }
                for dc in dcs:
                    d0 = (dch * 4 + dc) * 512
                    nc.tensor.matmul(
                        psum2[dc][:],
                        tT[b][RC - 1][:, row0:row0 + 128],
                        at_sb[RC - 1][:, d0:d0 + 512],
                        start=False,
                        stop=True,
                    )
                    j, half = dc // 2, dc % 2
                    nc.vector.tensor_add(
                        ot[j][:, half * 512:(half + 1) * 512],
                        psum2[dc][:],
                        bias_bc[:, d0:d0 + 512],
                    )
                    if fine_stores:
                        _store(
                            out_d[
                                b * BROWS + row0:b * BROWS + row0 + 128,
                                d0:d0 + 512,
                            ],
                            ot[j][:, half * 512:(half + 1) * 512],
                            alternate=True,
                        )
                    elif half == 1:
                        _store(
                            out_d[
                                b * BROWS + row0:b * BROWS + row0 + 128,
                                dch * 2048 + j * 1024:dch * 2048 + (j + 1) * 1024,
                            ],
                            ot[j][:],
                        )

            warmup()

            # stage1 block 0 (B + x co-stream); small PE fillers bridge
            # wire jitter at the early chunk boundaries
            ps_a = stage1_psum(0)
            for g in range(KB):
                stage1_group(0, g, ps_a)
                if g < 2:
                    filler(2)
                elif g < 4:
                    filler(1)
            load_bias()

            # stage1 block 1 first half (x(b1) g0..g3 ahead of A.T on the wire)
            ps_b = stage1_psum(1)
            for g in range(4):
                stage1_group(1, g, ps_b)

            # stage2 block 0, dch=0 units (A.T half0 lands behind x(b1) g0..g3)
            load_at_half(0)
            for rc2 in range(MB2):
                stage2_unit(0, rc2, 0)

            # stage1 block 1 second half
            for g in range(4, KB):
                stage1_group(1, g, ps_b)

            # stage2 block 0, dch=1 units
            load_at_half(1)
            for rc2 in range(MB2):
                stage2_unit(0, rc2, 1)

            # stage2 block 1; the very last unit is split into 2+2 psums so
            # only one evacuation remains after the final matmul
            for rc2 in range(MB2):
                for dch in range(2):
                    if rc2 == MB2 - 1 and dch == 1:
                        stage2_unit(1, rc2, dch, dcs=(0, 1), fine_stores=True)
                        stage2_unit(1, rc2, dch, dcs=(2, 3), fine_stores=True)
                    else:
                        stage2_unit(1, rc2, dch)

    nc.compile()
    return nc


def _get_nc():
    if "nc" not in _compiled:
        _compiled["nc"] = _build()
    return _compiled["nc"]


def run(inputs, trace=False, trace_kwargs=None):
    """Shard, execute on 8 cores, gather. Returns (output, BassKernelResults)."""
    x = np.asarray(inputs["x"], dtype=np.float32)
    A = np.asarray(inputs["A"], dtype=np.float32)
    B = np.asarray(inputs["B"], dtype=np.float32)
    bias = np.asarray(inputs["bias"], dtype=np.float32)

    x_flat = x.reshape(ROWS_TOTAL, D_IN)
    B_bf = B.astype(ml_dtypes.bfloat16)
    AT_bf = np.ascontiguousarray(A.T).astype(ml_dtypes.bfloat16)
    bias_bc = np.broadcast_to(
        bias.astype(ml_dtypes.bfloat16)[None, :], (128, D_OUT)
    ).copy()
    in_maps = []
    for i in range(N_CORES):
        xT_i = np.ascontiguousarray(x_flat[i * ROWS:(i + 1) * ROWS].T).astype(
            ml_dtypes.bfloat16
        )
        in_maps.append({"xT": xT_i, "b": B_bf, "at": AT_bf, "bias": bias_bc})

    nc = _get_nc()
    kwargs = {}
    if trace:
        kwargs["trace"] = True
        kwargs["trace_kwargs"] = trace_kwargs or {}
    res = None
    for attempt in range(3):
        try:
            res = run_bass_kernel_spmd(
                nc, in_maps, core_ids=list(range(N_CORES)), **kwargs
            )
        except Exception:
            # transient device/runtime hiccup; retry
            if attempt == 2:
                raise
            continue
        out = np.concatenate(
            [
                res.results[i]["out"].astype(np.float32)
                for i in range(N_CORES)
            ],
            axis=0,
        )
        if np.isfinite(out).all():
            return out.reshape(BATCH, SEQ, D_OUT), res
    return out.reshape(BATCH, SEQ, D_OUT), res


def kernel(**inputs) -> np.ndarray:
    out, _ = run(inputs)
    return out


# revision 42
# speedup vs baseline: 1.0233x; 1.0232x over previous
"""TRN2 Bass kernel for CompressedLinearLayer: out = x @ (A @ B.T).T + bias.

Computed low-rank: t = x @ B  (rank 512), out = t @ A.T + bias.
Sharding: data-parallel over the 8192 rows of x (1024 rows per core);
B, A.T, bias replicated. No collectives.

Device layouts (per core), bf16 on the wire for matmul inputs:
  xT   [4096, 1024] bf16  x rows shard, transposed+converted on host
  b    [4096, 512]  bf16  B
  at   [512, 4096]  bf16  A.T
  bias [4096]       f32
  out  [1024, 4096] bf16  natural orientation (host upconverts to f32)

Schedule (per core), rows in 2 blocks of 512:
  PE order: warmup -> s1(b0) g0..g7 -> s1(b1) g0..g3 -> s2(b0) dch0 units
            -> s1(b1) g4..g7 -> s2(b0) dch1 units -> s2(b1) all units.
  DMA wire order on the sync ring: x(b0) g0..g7, x(b1) g0..g3,
  A.T half0, x(b1) g4..g7, A.T half1.  B co-streams on the vector ring
  during phase A so each stage-1 group's B and x chunks land together.
  Output stores alternate between the scalar and gpsimd rings.
  Stage-2 units do the last k-chunk dc-major with immediate per-psum
  bias-add evacuation so PSUM bank turnaround hides behind the next
  unit's matmuls.  Accumulation is fp32 in PSUM; bias is added during
  PSUM evacuation on the vector engine; output stored as bf16.
"""
import numpy as np
import ml_dtypes

import concourse.bacc as bacc
import concourse.mybir as mybir
import concourse.tile as tile
from concourse.bass_utils import run_bass_kernel_spmd

N_CORES = 8
BATCH, SEQ = 4, 2048
D_IN, D_OUT, RANK = 4096, 4096, 512
ROWS_TOTAL = BATCH * SEQ           # 8192
ROWS = ROWS_TOTAL // N_CORES       # 1024 rows per core

F32 = mybir.dt.float32
BF16 = mybir.dt.bfloat16

KC = D_IN // 128     # 32 contraction chunks, stage 1
KSUB = 4             # k-chunks packed per DMA (0.5MB bf16 transfers)
KB = KC // KSUB      # 8 packed k-groups per block
RC = RANK // 128     # 4 rank chunks
NBLK = 2             # row blocks per core
BROWS = ROWS // NBLK # 512 rows per block
MB2 = BROWS // 128   # 4 row chunks of 128 per block (stage-2 out partitions)
N_WARM = 14          # PE warmup matmuls (ramp pstate during first DMA wait)

_compiled = {}


def _build():
    nc = bacc.Bacc("TRN2", target_bir_lowering=False, debug=False)

    xT_d = nc.declare_dram_parameter("xT", [D_IN, ROWS], BF16, isOutput=False)
    b_d = nc.declare_dram_parameter("b", [D_IN, RANK], BF16, isOutput=False)
    at_d = nc.declare_dram_parameter("at", [RANK, D_OUT], BF16, isOutput=False)
    bias_d = nc.declare_dram_parameter("bias", [128, D_OUT], BF16, isOutput=False)
    out_d = nc.declare_dram_parameter("out", [ROWS, D_OUT], BF16, isOutput=True)

    with tile.TileContext(nc) as tc:
        with (
            tc.tile_pool(name="wb", bufs=1) as wb,
            tc.tile_pool(name="xp", bufs=8) as xp,
            tc.tile_pool(name="tt", bufs=1) as ttp,
            tc.tile_pool(name="op", bufs=6) as op,
            tc.tile_pool(name="ps1", bufs=4, space="PSUM") as ps1p,
            tc.tile_pool(name="ps2", bufs=4, space="PSUM") as ps2p,
        ):
            bias_bc = wb.tile([128, D_OUT], BF16, tag="bias_bc")

            # B resident: 8 tiles [128, 4, 512] bf16 = 0.5MB each
            b_sb = [
                wb.tile([128, KSUB, RANK], BF16, tag=f"b{g}", name=f"b{g}")
                for g in range(KB)
            ]
            # A.T resident: 4 tiles [128, 4096] bf16 (1MB each)
            at_sb = [
                wb.tile([128, D_OUT], BF16, tag=f"at{r}", name=f"at{r}")
                for r in range(RC)
            ]
            # tT per block: 4 tiles [128, 512] bf16 each
            tT = [
                [
                    ttp.tile([128, BROWS], BF16, tag=f"tT{b}_{r}", name=f"tT{b}_{r}")
                    for r in range(RC)
                ]
                for b in range(NBLK)
            ]
            # warmup scratch
            wstat = wb.tile([128, 128], BF16, tag="wstat")
            wmov = wb.tile([128, 512], BF16, tag="wmov")
            wps_ref = []

            def warmup():
                # parallel engines so the warmup LDWEIGHTS isn't gated on a
                # serialized memset chain
                nc.vector.memset(wstat[:], 0.0)
                nc.vector.memset(wmov[:], 0.0)
                wps = ps2p.tile([128, 512], F32, tag="ps2", name="warm_ps")
                wps_ref.append(wps)
                for i in range(N_WARM):
                    nc.tensor.matmul(
                        wps[:], wstat[:], wmov[:], start=True, stop=True
                    )

            def filler(n):
                # keep the PE busy (and its pstate up) across a DMA-arrival
                # jitter window; results are never read
                wps = wps_ref[0]
                for i in range(n):
                    nc.tensor.matmul(
                        wps[:], wstat[:], wmov[:], start=True, stop=True
                    )

            def load_bias():
                # bias pre-replicated to 128 partitions on the host (bf16);
                # rides the scalar ring behind B, needed only from ~50us
                nc.scalar.dma_start(bias_bc[:], bias_d[:])

            def stage1_group(b, g, psum1):
                # DMAs: x chunk on sync ring; B chunk (block 0 only) on
                # vector ring so both streams co-arrive in phase A.
                # phase-A arrival granularity: quarter chunks while the PE
                # has no DMA lead yet, coarsening as the lead builds
                split = 2 if (b == 0 and g < 3) else 1
                xq = nc.sync
                xg = xp.tile([128, KSUB, BROWS], BF16, tag="xk", name=f"x{b}_{g}")
                for sp in range(split):
                    lo, hi = sp * KSUB // split, (sp + 1) * KSUB // split
                    if b == 0:
                        nc.scalar.dma_start(
                            b_sb[g][:, lo:hi, :],
                            b_d[(g * KSUB + lo) * 128:(g * KSUB + hi) * 128, :]
                            .rearrange("(ks p) r -> p ks r", p=128),
                        )
                    xq.dma_start(
                        xg[:, lo:hi, :],
                        xT_d[
                            (g * KSUB + lo) * 128:(g * KSUB + hi) * 128,
                            b * BROWS:(b + 1) * BROWS,
                        ].rearrange("(ks p) m -> p ks m", p=128),
                    )
                last = g == KB - 1
                if not last:
                    for ks in range(KSUB):
                        k = g * KSUB + ks
                        for mc in range(RC):
                            nc.tensor.matmul(
                                psum1[mc][:],
                                b_sb[g][:, ks, mc * 128:(mc + 1) * 128],
                                xg[:, ks, :],
                                start=(k == 0),
                                stop=False,
                            )
                else:
                    # invert loops so each psum finishes (and can evacuate to
                    # tT on the DVE) while the PE continues with the next mc
                    for mc in range(RC):
                        for ks in range(KSUB):
                            nc.tensor.matmul(
                                psum1[mc][:],
                                b_sb[g][:, ks, mc * 128:(mc + 1) * 128],
                                xg[:, ks, :],
                                start=False,
                                stop=(ks == KSUB - 1),
                            )
                        nc.scalar.activation(
                            tT[b][mc][:], psum1[mc][:],
                            mybir.ActivationFunctionType.Copy,
                        )

            def stage1_psum(b):
                return [
                    ps1p.tile([128, BROWS], F32, tag="ps1", name=f"ps1_{b}_{i}")
                    for i in range(RC)
                ]

            def load_at_half(h):
                # sync ring: wire-ordered behind the x chunks emitted so far
                for r in range(RC):
                    nc.sync.dma_start(
                        at_sb[r][:, h * 2048:(h + 1) * 2048],
                        at_d[r * 128:(r + 1) * 128, h * 2048:(h + 1) * 2048],
                    )

            _store_flip = [0]

            def _store(dram_ap, sbuf_ap, alternate=False):
                eng = nc.scalar
                if alternate:
                    # sync ring is idle by the time the final stores go out
                    eng = nc.scalar if _store_flip[0] == 0 else nc.sync
                    _store_flip[0] ^= 1
                eng.dma_start(dram_ap, sbuf_ap)

            def stage2_unit(b, rc2, dch, dcs=(0, 1, 2, 3), fine_stores=False):
                row0 = rc2 * 128
                psum2 = {
                    dc: ps2p.tile(
                        [128, 512], F32, tag="ps2",
                        name=f"ps2_{b}_{rc2}_{dch}_{dc}",
                    )
                    for dc in dcs
                }
                # k = 0: dc-inner
                for dc in dcs:
                    d0 = (dch * 4 + dc) * 512
                    nc.tensor.matmul(
                        psum2[dc][:],
                        tT[b][0][:, row0:row0 + 128],
                        at_sb[0][:, d0:d0 + 512],
                        start=True,
                        stop=False,
                    )
                # k = 1..3: dc-major with immediate evacuation, spacing the
                # DVE adds >= 3 matmuls apart so each psum bank frees before
                # the next unit's deadline for it
                ot = {
                    j: op.tile([128, 1024], BF16, tag="ot",
                               name=f"ot{b}_{rc2}_{dch}_{j}")
                    for j in set(dc // 2 for dc in dcs)
                }
                for dc in dcs:
                    d0 = (dch * 4 + dc) * 512
                    for k in range(1, RC):
                        nc.tensor.matmul(
                            psum2[dc][:],
                            tT[b][k][:, row0:row0 + 128],
                            at_sb[k][:, d0:d0 + 512],
                            start=False,
                            stop=(k == RC - 1),
                        )
                    j, half = dc // 2, dc % 2
                    nc.vector.tensor_add(
                        ot[j][:, half * 512:(half + 1) * 512],
                        psum2[dc][:],
                        bias_bc[:, d0:d0 + 512],
                    )
                    if fine_stores:
                        _store(
                            out_d[
                                b * BROWS + row0:b * BROWS + row0 + 128,
                                d0:d0 + 512,
                            ],
                            ot[j][:, half * 512:(half + 1) * 512],
                            alternate=True,
                        )
                    elif half == 1:
                        _store(
                            out_d[
                                b * BROWS + row0:b * BROWS + row0 + 128,
                                dch * 2048 + j * 1024:dch * 2048 + (j + 1) * 1024,
                            ],
                            ot[j][:],
                        )

            warmup()

            # stage1 block 0 (B + x co-stream); small PE fillers bridge
            # wire jitter at the early chunk boundaries
            ps_a = stage1_psum(0)
            for g in range(KB):
                stage1_group(0, g, ps_a)
                if g < 2:
                    filler(2)
                elif g < 4:
                    filler(1)
            load_bias()

            # stage1 block 1 first half (x(b1) g0..g3 ahead of A.T on the wire)
            ps_b = stage1_psum(1)
            for g in range(4):
                stage1_group(1, g, ps_b)

            # stage2 block 0, dch=0 units (A.T half0 lands behind x(b1) g0..g3)
            load_at_half(0)
            for rc2 in range(MB2):
                stage2_unit(0, rc2, 0)

            # stage1 block 1 second half
            for g in range(4, KB):
                stage1_group(1, g, ps_b)

            # stage2 block 0, dch=1 units
            load_at_half(1)
            for rc2 in range(MB2):
                stage2_unit(0, rc2, 1)

            # stage2 block 1; the very last unit is split into 2+2 psums so
            # only one evacuation remains after the final matmul
            for rc2 in range(MB2):
                for dch in range(2):
                    if rc2 == MB2 - 1 and dch == 1:
                        stage2_unit(1, rc2, dch, dcs=(0, 1), fine_stores=True)
                        stage2_unit(1, rc2, dch, dcs=(2, 3), fine_stores=True)
                    else:
                        stage2_unit(1, rc2, dch)

    nc.compile()
    return nc


def _get_nc():
    if "nc" not in _compiled:
        _compiled["nc"] = _build()
    return _compiled["nc"]


def run(inputs, trace=False, trace_kwargs=None):
    """Shard, execute on 8 cores, gather. Returns (output, BassKernelResults)."""
    x = np.asarray(inputs["x"], dtype=np.float32)
    A = np.asarray(inputs["A"], dtype=np.float32)
    B = np.asarray(inputs["B"], dtype=np.float32)
    bias = np.asarray(inputs["bias"], dtype=np.float32)

    x_flat = x.reshape(ROWS_TOTAL, D_IN)
    B_bf = B.astype(ml_dtypes.bfloat16)
    AT_bf = np.ascontiguousarray(A.T).astype(ml_dtypes.bfloat16)
    bias_bc = np.broadcast_to(
        bias.astype(ml_dtypes.bfloat16)[None, :], (128, D_OUT)
    ).copy()
    in_maps = []
    for i in range(N_CORES):
        xT_i = np.ascontiguousarray(x_flat[i * ROWS:(i + 1) * ROWS].T).astype(
            ml_dtypes.bfloat16
        )
        in_maps.append({"xT": xT_i, "b": B_bf, "at": AT_bf, "bias": bias_bc})

    nc = _get_nc()
    kwargs = {}
    if trace:
        kwargs["trace"] = True
        kwargs["trace_kwargs"] = trace_kwargs or {}
    res = None
    for attempt in range(3):
        try:
            res = run_bass_kernel_spmd(
                nc, in_maps, core_ids=list(range(N_CORES)), **kwargs
            )
        except Exception:
            # transient device/runtime hiccup; retry
            if attempt == 2:
                raise
            continue
        out = np.concatenate(
            [
                res.results[i]["out"].astype(np.float32)
                for i in range(N_CORES)
            ],
            axis=0,
        )
        if np.isfinite(out).all():
            return out.reshape(BATCH, SEQ, D_OUT), res
    return out.reshape(BATCH, SEQ, D_OUT), res


def kernel(**inputs) -> np.ndarray:
    out, _ = run(inputs)
    return out
